# revision 1
# baseline (speedup 1.0000x reference)
"""Trainium2 Bass kernel for nn_Attention_10771777978404 (sparse_attention).

Sharding over 8 NeuronCores: 2 batch-groups x 4 cores (tensor parallel over
heads within each batch group).
  - core ci handles batch ci//4 and heads [4*(ci%4), 4*(ci%4)+4): it computes
    its q/k/v projections (columns of wq/wk/wv), RoPE, causal attention with
    the low-rank sigmoid gate, and a full-width partial of the output
    projection from its 4 heads (rows of wo).
  - the rank-32 adapter weights are replicated inside each batch group; the
    sigmoid gate is computed as 1/(1+exp(-A)) so the scalar engine only ever
    uses the Exp table (no Sigmoid-table reloads, no DRAM staging).
  - host sums the 4 partial output projections per batch (fp16 partials).

Everything on-device is bf16 with fp32 PSUM accumulation.

self-contained: hardcodes the problem shapes; only needs `concourse` (on
PYTHONPATH in this container) + jax axon devices.
"""

import math
from contextlib import ExitStack
from dataclasses import dataclass

import numpy as np
import ml_dtypes

import concourse.tile as tile
from concourse import bacc
from concourse import mybir
from concourse import bass_utils

BF16 = mybir.dt.bfloat16
F8 = mybir.dt.float8e4
DR = mybir.MatmulPerfMode.DoubleRow
WSCALE = 32.0  # fp8 weight prescale (keeps weights out of the subnormal range)
F16 = mybir.dt.float16
F32 = mybir.dt.float32
AF = mybir.ActivationFunctionType


@dataclass(frozen=True)
class Cfg:
    B: int = 2
    S: int = 2048
    DIM: int = 2048
    NH: int = 16
    HD: int = 128
    RANK: int = 32
    NCORES: int = 8
    GROUPS: int = 2     # batch groups of 4 cores
    QT: int = 512       # query block (free dim of score tiles)
    KT: int = 128       # key block (partition dim of score tiles)
    repeat: int = 1     # hardware-loop repetitions of the whole body (timing)
    # ablation flags (profiling on hardware; all True for the real kernel)
    use_gate: bool = True
    use_rowsum: bool = True
    use_attn: bool = True
    use_wo: bool = True
    # fp8e4m3 DoubleRow projections: ~1.6x faster projections in the sim but
    # costs 8e-2 relative error (peaked softmax amplifies logit noise) vs the
    # 2e-2 gate — keep False
    use_fp8: bool = False

    @property
    def CPG(self):
        return self.NCORES // self.GROUPS  # cores per batch group

    @property
    def HLOC(self):
        return self.NH // self.CPG  # heads per core (4)

    @property
    def DH(self):
        return self.HLOC * self.HD  # per-core head-dim span (512)

    @property
    def KTILES(self):
        return self.DIM // 128  # contraction tiles for projections

    @property
    def QTN(self):
        return self.S // self.QT

    @property
    def DIAG(self):
        return self.QT // self.KT  # k-tiles per diagonal band

    @property
    def KP(self):
        return self.KTILES // 2  # DoubleRow contraction pairs


FULL = Cfg()


def build_nc(cfg: Cfg = FULL):
    c = cfg
    assert c.HD == 128 and c.KT == 128
    nc = bacc.Bacc("TRN2", target_bir_lowering=False, debug=False,
                   num_devices=c.NCORES)

    # ---- kernel I/O (per core: one batch, HLOC heads) ----
    PDT = F8 if c.use_fp8 else BF16  # projection operand dtype
    xT = nc.dram_tensor("xT", [c.DIM, c.S], PDT, kind="ExternalInput")
    wqT = nc.dram_tensor("wqT", [c.DIM, c.DH], PDT, kind="ExternalInput")
    wkT = nc.dram_tensor("wkT", [c.DIM, c.DH], PDT, kind="ExternalInput")
    wvT = nc.dram_tensor("wvT", [c.DIM, c.DH], PDT, kind="ExternalInput")
    wocT = nc.dram_tensor("wocT", [c.DH, c.DIM], BF16, kind="ExternalInput")
    waT = nc.dram_tensor("waT", [c.DIM, 2 * c.RANK], PDT, kind="ExternalInput")
    c2d = nc.dram_tensor("c2d", [c.HD, c.S], BF16, kind="ExternalInput")
    s2d = nc.dram_tensor("s2d", [c.HD, c.S], BF16, kind="ExternalInput")
    pswapd = nc.dram_tensor("pswapd", [c.HD, c.HD], BF16, kind="ExternalInput")
    maskdd = nc.dram_tensor("maskdd", [c.DIAG, c.KT, c.QT], BF16, kind="ExternalInput")

    # partial output projection, transposed: pout[j, t] (fp16; host sums the
    # 4 partials of each batch group in fp32)
    pout = nc.dram_tensor("pout", [c.DIM, c.S], F16, kind="ExternalOutput")

    isqrt = 1.0 / math.sqrt(c.HD)
    # fp8 path: q,k,v,aq,ak all carry a WSCALE factor from the prescaled
    # weights; compensate in the exp scales (scores, gate) and on the host (v)
    wsq = WSCALE * WSCALE if c.use_fp8 else 1.0
    sc_score = isqrt / wsq
    sc_gate = -1.0 / wsq
    NKT = c.KP if c.use_fp8 else c.KTILES
    pmode = DR if c.use_fp8 else None

    with ExitStack() as _ctx:
        tc = _ctx.enter_context(tile.TileContext(nc))
        # persistent pools (whole-iteration lifetime)
        cst = _ctx.enter_context(tc.tile_pool(name="const", bufs=1))
        adp = _ctx.enter_context(tc.tile_pool(name="ap", bufs=1))
        qkp = _ctx.enter_context(tc.tile_pool(name="qk", bufs=1))
        vp = _ctx.enter_context(tc.tile_pool(name="vp", bufs=1))
        rtp = _ctx.enter_context(tc.tile_pool(name="rope_t", bufs=1))
        pge = _ctx.enter_context(tc.tile_pool(name="pge", bufs=5))
        gwk = _ctx.enter_context(tc.tile_pool(name="gwk", bufs=2))
        ogp = _ctx.enter_context(tc.tile_pool(name="og", bufs=1))
        wop = _ctx.enter_context(tc.tile_pool(name="wo_out", bufs=2))
        # PSUM pools (8 banks total): pp 2 + ps/pga/rbc 3 + po 2 + prs 1
        pp = _ctx.enter_context(tc.tile_pool(name="pp", bufs=2, space="PSUM"))
        psp = _ctx.enter_context(tc.tile_pool(name="ps", bufs=3, space="PSUM"))
        pop = _ctx.enter_context(tc.tile_pool(name="po", bufs=2, space="PSUM"))
        prsp = _ctx.enter_context(tc.tile_pool(name="prs", bufs=1, space="PSUM"))

        def body():
            # ---- constants ----
            c2_sb = cst.tile([128, c.S], BF16, name="c2_sb", tag="c2")
            s2_sb = cst.tile([128, c.S], BF16, name="s2_sb", tag="s2")
            psw_sb = cst.tile([128, 128], BF16, name="psw_sb", tag="psw")
            mask_sb = cst.tile([128, c.DIAG, c.QT], BF16, name="mask_sb", tag="mask")
            ones_sb = cst.tile([128, 1], BF16, name="ones_sb", tag="ones")
            oner_sb = cst.tile([1, 128], F16, name="oner_sb", tag="oner")

            # packed adapter projections: one [64,512] matmul computes both
            # aq and ak rows (halves the adapter matmul count); ak is then
            # moved to partition base 0 with one SBUF->SBUF DMA (engines
            # cannot shift partitions, DMA can)
            aqk_sb = adp.tile([2 * c.RANK, c.S], BF16, name="aqk_sb", tag="aqk")
            ak_sb = adp.tile([c.RANK, c.S], BF16, name="ak_sb", tag="ak")
            q_sb = [qkp.tile([128, c.S], BF16, name=f"q{h}_sb", tag=f"q{h}")
                    for h in range(c.HLOC)]
            k_sb = [qkp.tile([128, c.S], BF16, name=f"k{h}_sb", tag=f"k{h}")
                    for h in range(c.HLOC)]
            v_sb = vp.tile([128, c.S // 128, c.DH], BF16, name="v_sb", tag="v")

            with tc.tile_pool(name="xtp", bufs=1) as xtp:
                if c.use_fp8:
                    wa_sb = xtp.tile([128, NKT, 2, 2 * c.RANK], F8,
                                     name="wa_sb", tag="wa")
                    nc.sync.dma_start(
                        out=wa_sb,
                        in_=waT.ap().rearrange("(t two p) m -> p t two m",
                                               p=128, two=2))
                    xt_sb = xtp.tile([128, NKT, 2, c.S], F8, name="xt_sb", tag="xt")
                    xr = xT.ap().rearrange("(t two p) n -> p t two n", p=128, two=2)
                    for kt in range(NKT):
                        nc.sync.dma_start(out=xt_sb[:, kt, :, :], in_=xr[:, kt, :, :])
                else:
                    wa_sb = xtp.tile([128, c.KTILES, 2 * c.RANK], BF16,
                                     name="wa_sb", tag="wa")
                    nc.sync.dma_start(out=wa_sb,
                                      in_=waT.ap().rearrange("(t p) m -> p t m", p=128))
                    xt_sb = xtp.tile([128, c.KTILES, c.S], BF16, name="xt_sb", tag="xt")
                    xr = xT.ap().rearrange("(t p) n -> p t n", p=128)
                    for kt in range(c.KTILES):
                        nc.sync.dma_start(out=xt_sb[:, kt, :], in_=xr[:, kt, :])

                def xsl(j, sl):
                    return xt_sb[:, j, :, sl] if c.use_fp8 else xt_sb[:, j, sl]

                def wsl(w, j, sl):
                    return w[:, j, :, sl] if c.use_fp8 else w[:, j, sl]
                nc.sync.dma_start(out=c2_sb, in_=c2d.ap())
                nc.sync.dma_start(out=s2_sb, in_=s2d.ap())
                nc.sync.dma_start(out=psw_sb, in_=pswapd.ap())
                nc.sync.dma_start(out=mask_sb,
                                  in_=maskdd.ap().rearrange("j p q -> p j q"))
                nc.vector.memset(ones_sb, 1.0)
                nc.vector.memset(oner_sb, 1.0)

                # ---- adapter projections: aq, ak [RANK, S] ----
                # kt-outer with 8 live accumulators (borrowed across the four
                # PSUM pools) so the PE issues 8 matmuls per arriving x chunk
                # instead of idling at DMA pace
                apools = [pp, psp, pop, prsp]
                acc = [apools[i].tile([2 * c.RANK, c.QT], F32, name=f"acc_a{i}",
                                      tag=("pp", "ps", "po", "prs")[i])
                       for i in range(c.QTN)]

                # ---- q/k projections ----
                with tc.tile_pool(name="wqk", bufs=1) as wp:
                    if c.use_fp8:
                        wq_sb = wp.tile([128, NKT, 2, c.DH], F8, name="wq_sb", tag="wq")
                        wk_sb = wp.tile([128, NKT, 2, c.DH], F8, name="wk_sb", tag="wk")
                        for w_sb, w_d in ((wq_sb, wqT), (wk_sb, wkT)):
                            nc.scalar.dma_start(
                                out=w_sb,
                                in_=w_d.ap().rearrange("(t two p) m -> p t two m",
                                                       p=128, two=2))
                    else:
                        wq_sb = wp.tile([128, c.KTILES, c.DH], BF16, name="wq_sb", tag="wq")
                        wk_sb = wp.tile([128, c.KTILES, c.DH], BF16, name="wk_sb", tag="wk")
                        for w_sb, w_d in ((wq_sb, wqT), (wk_sb, wkT)):
                            wr = w_d.ap().rearrange("(t p) m -> p t m", p=128)
                            for half in range(2):
                                h0 = half * (c.KTILES // 2)
                                nc.scalar.dma_start(out=w_sb[:, h0:h0 + c.KTILES // 2, :],
                                                    in_=wr[:, h0:h0 + c.KTILES // 2, :])
                    allqk = [(dst, w, h, qt)
                             for dst, w in ((q_sb, wq_sb), (k_sb, wk_sb))
                             for h in range(c.HLOC)
                             for qt in range(c.QTN)]
                    # first four groups run kt-outer interleaved with the
                    # adapter accumulation (emitted above) so the PE issues
                    # 8 matmuls per arriving x chunk instead of 4
                    head_pools = [(pp, "pp"), (psp, "ps"), (psp, "ps"),
                                  (pop, "po")]
                    headacc = [head_pools[i][0].tile([128, c.QT], F32,
                                                     name=f"acc_qk{i}",
                                                     tag=head_pools[i][1])
                               for i in range(4)]
                    for kt in range(NKT):
                        for qt in range(c.QTN):
                            nc.tensor.matmul(
                                acc[qt][:, :],
                                wsl(wa_sb, kt, slice(0, 2 * c.RANK)),
                                xsl(kt, slice(qt * c.QT, (qt + 1) * c.QT)),
                                start=(kt == 0), stop=(kt == NKT - 1),
                                perf_mode=pmode)
                        for i in range(4):
                            dst, w, h, qt = allqk[i]
                            nc.tensor.matmul(
                                headacc[i][:, :],
                                wsl(w, kt, slice(h * 128, (h + 1) * 128)),
                                xsl(kt, slice(qt * c.QT, (qt + 1) * c.QT)),
                                start=(kt == 0), stop=(kt == NKT - 1),
                                perf_mode=pmode)
                    for qt in range(c.QTN):
                        nc.vector.tensor_copy(
                            aqk_sb[:, qt * c.QT:(qt + 1) * c.QT], acc[qt][:, :])
                    nc.sync.dma_start(out=ak_sb[:, :],
                                      in_=aqk_sb[c.RANK:2 * c.RANK, :])
                    for i in range(4):
                        dst, w, h, qt = allqk[i]
                        nc.scalar.copy(dst[h][:, qt * c.QT:(qt + 1) * c.QT],
                                       headacc[i][:, :])
                    for dst, w, h, qt in allqk[4:]:
                        psum = pp.tile([128, c.QT], F32, name="psum_qk", tag="pp")
                        for kt in range(NKT):
                            nc.tensor.matmul(
                                psum[:, :],
                                wsl(w, kt, slice(h * 128, (h + 1) * 128)),
                                xsl(kt, slice(qt * c.QT, (qt + 1) * c.QT)),
                                start=(kt == 0), stop=(kt == NKT - 1),
                                perf_mode=pmode)
                        nc.scalar.copy(dst[h][:, qt * c.QT:(qt + 1) * c.QT],
                                       psum[:, :])

                # ---- v projection: [tok, d] natural, moving 512 wide ----
                # RoPE tiles are interleaved into the v loop: the rope chain
                # is DVE/GPSIMD-paced, the v matmuls keep the PE busy
                rope_tiles = [(tiles, h, qt)
                              for tiles in (q_sb, k_sb)
                              for h in range(c.HLOC)
                              for qt in range(c.QTN)]

                def rope_step(i):
                    # m1 on GPSIMD (SBUF-only engine), m2 on DVE (PSUM read),
                    # final add alternates so neither engine is the pacer
                    tiles, h, qt = rope_tiles[i]
                    eng = nc.vector if i % 2 == 0 else nc.gpsimd
                    sl = slice(qt * c.QT, (qt + 1) * c.QT)
                    pswp = psp.tile([128, c.QT], F32, name="pswp", tag="ps")
                    nc.tensor.matmul(pswp[:, :], psw_sb[:, :],
                                     tiles[h][:, sl], start=True, stop=True)
                    m1 = rtp.tile([128, c.QT], BF16, name="rope_m1",
                                  tag=f"m1{i % 2}")
                    m2 = rtp.tile([128, c.QT], BF16, name="rope_m2",
                                  tag=f"m2{i % 2}")
                    nc.gpsimd.tensor_mul(m1[:, :], tiles[h][:, sl], c2_sb[:, sl])
                    nc.vector.tensor_mul(m2[:, :], pswp[:, :], s2_sb[:, sl])
                    eng.tensor_add(tiles[h][:, sl], m1[:, :], m2[:, :])

                with tc.tile_pool(name="wvp", bufs=1) as wvpool:
                    if c.use_fp8:
                        wv_sb = wvpool.tile([128, NKT, 2, c.DH], F8,
                                            name="wv_sb", tag="wv")
                        nc.sync.dma_start(
                            out=wv_sb,
                            in_=wvT.ap().rearrange("(t two p) m -> p t two m",
                                                   p=128, two=2))
                    else:
                        wv_sb = wvpool.tile([128, c.KTILES, c.DH], BF16,
                                            name="wv_sb", tag="wv")
                        wr = wvT.ap().rearrange("(t p) m -> p t m", p=128)
                        for half in range(2):
                            h0 = half * (c.KTILES // 2)
                            nc.sync.dma_start(out=wv_sb[:, h0:h0 + c.KTILES // 2, :],
                                              in_=wr[:, h0:h0 + c.KTILES // 2, :])
                    for tt in range(c.S // 128):
                        psum = pp.tile([128, c.DH], F32, name="psum_v", tag="pp")
                        for kt in range(NKT):
                            nc.tensor.matmul(
                                psum[:, :],
                                xsl(kt, slice(tt * 128, (tt + 1) * 128)),
                                wsl(wv_sb, kt, slice(0, c.DH)),
                                start=(kt == 0), stop=(kt == NKT - 1),
                                perf_mode=pmode)
                        nc.scalar.copy(v_sb[:, tt, :], psum[:, :])
                        rope_step(2 * tt)
                        rope_step(2 * tt + 1)

            # ---- attention + output projection, per 512-query block ----
            with tc.tile_pool(name="wog", bufs=1) as wog:
                woc_sb = wog.tile([128, c.HLOC, c.DIM], BF16, name="woc_sb", tag="woc")
                wcr = wocT.ap().rearrange("(h p) j -> p h j", p=128)
                for h in range(c.HLOC):
                    nc.sync.dma_start(out=woc_sb[:, h, :], in_=wcr[:, h, :])
                # double-buffered per-qt gate tiles r = sigmoid(A)[k, q]
                rg_sb = [wog.tile([128, c.DIAG * c.QTN, c.QT], BF16,
                                  name=f"rg{i}_sb", tag=f"rg{i}") for i in range(2)]

                def gate_step(qt, kt):
                    # r = 1/(1+exp(-A)) via the (shared) Exp table
                    qsl = slice(qt * c.QT, (qt + 1) * c.QT)
                    ksl = slice(kt * c.KT, (kt + 1) * c.KT)
                    pga = psp.tile([128, c.QT], F32, name="pga", tag="ps")
                    nc.tensor.matmul(pga[:, :], ak_sb[:, ksl],
                                     aqk_sb[0:c.RANK, qsl],
                                     start=True, stop=True)
                    ge = pge.tile([128, c.QT], BF16, name="ge", tag="p")
                    nc.scalar.activation(ge[:, :], pga[:, :], AF.Exp, scale=sc_gate)
                    gt = gwk.tile([128, c.QT], F32, name="gt", tag="gt")
                    nc.vector.tensor_scalar_add(gt[:, :], ge[:, :], 1.0)
                    gr = gwk.tile([128, c.QT], F32, name="gr", tag="gr")
                    nc.vector.reciprocal_approx_fast(out=gr[:, :], in_=gt[:, :])
                    nc.gpsimd.tensor_copy(rg_sb[qt % 2][:, kt, :], gr[:, :])

                if c.use_gate and c.use_attn:
                    for kt in range(c.DIAG):
                        gate_step(0, kt)

                for qt in range(c.QTN):
                    qsl = slice(qt * c.QT, (qt + 1) * c.QT)
                    nkt = c.DIAG * (qt + 1)  # causal k tiles
                    rg = rg_sb[qt % 2]

                    og_sb = ogp.tile([128, c.HLOC, c.QT], BF16, name="og_sb", tag="og")
                    if not c.use_attn:
                        nc.vector.memset(og_sb, 0.0)
                    # normalize chain of head h is emitted early in head h+1's
                    # score phase so its latency hides behind score matmuls
                    pending_norm = [None]

                    def make_normalize(h, po, prs):
                        def norm():
                            if not c.use_rowsum:
                                nc.vector.tensor_copy(og_sb[:, h, :], po[:, :])
                                return
                            rr = gwk.tile([1, c.QT], F32, name="rr", tag="rr")
                            nc.vector.reciprocal_approx_fast(out=rr[:, :],
                                                             in_=prs[:, :])
                            rrh = gwk.tile([1, c.QT], F16, name="rrh", tag="rrh")
                            nc.vector.tensor_copy(rrh[:, :], rr[:, :])
                            rbc = psp.tile([128, c.QT], F32, name="rbc", tag="ps")
                            nc.tensor.matmul(rbc[:, :], oner_sb[:, :], rrh[:, :],
                                             start=True, stop=True)
                            # DVE can't take two PSUM operands; stage the
                            # broadcast in SBUF via ACT
                            rbs = gwk.tile([128, c.QT], F16, name="rbs", tag="rbs")
                            nc.scalar.copy(rbs[:, :], rbc[:, :])
                            nc.vector.tensor_mul(og_sb[:, h, :], po[:, :],
                                                 rbs[:, :])
                        return norm

                    for h in range(c.HLOC if c.use_attn else 0):
                        po = pop.tile([128, c.QT], F32, name="po", tag="po")
                        prs = prsp.tile([1, c.QT], F32, name="prs", tag="prs")

                        # score matmuls run PIPE steps ahead of the dependent
                        # rowsum/AV matmuls so the exp/mask/gate chain latency
                        # stays off the in-order PE queue
                        PIPE = 3
                        stage = []  # (kt, p_or_pm_tile, pgm_tile)

                        def score_step(kt):
                            ksl = slice(kt * c.KT, (kt + 1) * c.KT)
                            ps = psp.tile([128, c.QT], F32, name="ps", tag="ps")
                            nc.tensor.matmul(ps[:, :], k_sb[h][:, ksl],
                                             q_sb[h][:, qsl], start=True, stop=True)
                            p_sb = pge.tile([128, c.QT], BF16, name="p_sb", tag="p")
                            nc.scalar.activation(p_sb[:, :], ps[:, :], AF.Exp,
                                                 scale=sc_score)
                            j = kt - c.DIAG * qt
                            if j >= 0:
                                # diagonal band: 0/1 causal mask after exp
                                pm = pge.tile([128, c.QT], BF16, name="pm", tag="pm")
                                nc.vector.tensor_mul(pm[:, :], p_sb[:, :],
                                                     mask_sb[:, j, :])
                                p_sb = pm
                            if c.use_gate:
                                pgm = pge.tile([128, c.QT], BF16, name="pgm",
                                               tag="pgm")
                                nc.vector.tensor_mul(pgm[:, :], p_sb[:, :],
                                                     rg[:, kt, :])
                            else:
                                pgm = p_sb
                            stage.append((kt, p_sb, pgm))

                        def drain_step():
                            kt, p_sb, pgm = stage.pop(0)
                            # pre-gate rowsum (softmax denominator)
                            if c.use_rowsum:
                                nc.tensor.matmul(prs[:, :], ones_sb[:, :],
                                                 p_sb[:, :],
                                                 start=(kt == 0),
                                                 stop=(kt == nkt - 1))
                            # out_h^T[d, q] += v[k, d].T @ p_gated[k, q]
                            nc.tensor.matmul(po[:, :],
                                             v_sb[:, kt, h * 128:(h + 1) * 128],
                                             pgm[:, :],
                                             start=(kt == 0), stop=(kt == nkt - 1))

                        for kt in range(nkt):
                            score_step(kt)
                            if kt == min(nkt - 1, 7) and pending_norm[0] is not None:
                                pending_norm[0]()
                                pending_norm[0] = None
                            if len(stage) > PIPE:
                                drain_step()
                        while stage:
                            drain_step()
                        assert pending_norm[0] is None, "normalize dropped"
                        pending_norm[0] = make_normalize(h, po, prs)
                    # last head's normalize must land before the wo matmuls
                    if pending_norm[0] is not None:
                        pending_norm[0]()

                    # output-projection partial for this query block,
                    # interleaved with the NEXT block's gate generation so the
                    # PE fills the gate chain's latency with wo matmuls
                    nkt2 = (c.DIAG * (qt + 2)
                            if (qt + 1 < c.QTN and c.use_gate and c.use_attn)
                            else 0)
                    for ch in range(c.DIM // 128 if c.use_wo else 0):
                        pf = pp.tile([128, c.QT], F32, name="pf", tag="pp")
                        for h in range(c.HLOC):
                            nc.tensor.matmul(
                                pf[:, :],
                                woc_sb[:, h, ch * 128:(ch + 1) * 128],
                                og_sb[:, h, :],
                                start=(h == 0), stop=(h == c.HLOC - 1))
                        f_sb = wop.tile([128, c.QT], F16, name="f_sb", tag="f")
                        nc.scalar.copy(f_sb[:, :], pf[:, :])
                        nc.sync.dma_start(
                            out=pout.ap()[ch * 128:(ch + 1) * 128, qsl],
                            in_=f_sb[:, :])
                        if ch < nkt2:
                            gate_step(qt + 1, ch)
                    if not c.use_wo:
                        for kt in range(nkt2):
                            gate_step(qt + 1, kt)

        if c.repeat > 1:
            with tc.For_i(0, c.repeat, 1,
                          hint_engines=(mybir.EngineType.PE,
                                        mybir.EngineType.DVE,
                                        mybir.EngineType.Activation,
                                        mybir.EngineType.Pool,
                                        mybir.EngineType.SP)):
                body()
        else:
            body()

    nc.compile()
    return nc


def make_core_inputs(inputs: dict, cfg: Cfg = FULL):
    """Host-side sharding: returns in_maps (one dict per core)."""
    c = cfg
    bf16 = ml_dtypes.bfloat16
    x = np.asarray(inputs["x"])
    mask = np.asarray(inputs["mask"])
    fc = np.asarray(inputs["freqs_cos"])
    fs = np.asarray(inputs["freqs_sin"])
    wq, wk, wv, wo = (np.asarray(inputs[k]) for k in ("wq", "wk", "wv", "wo"))
    wa_q, wa_k = np.asarray(inputs["wa_q"]), np.asarray(inputs["wa_k"])

    import concourse.mybir as _mb
    pdt = _mb.dt.np(F8) if c.use_fp8 else bf16
    wsc = WSCALE if c.use_fp8 else 1.0
    xTb = [np.ascontiguousarray(x[b].T).astype(pdt) for b in range(c.B)]
    waT = np.ascontiguousarray(
        np.concatenate([wa_q, wa_k], axis=0).T * wsc).astype(pdt)

    # rope tables in [d, tok] layout
    c2 = np.empty((c.HD, c.S), np.float32)
    s2 = np.empty((c.HD, c.S), np.float32)
    c2[0::2] = fc.T
    c2[1::2] = fc.T
    s2[0::2] = -fs.T
    s2[1::2] = fs.T
    c2 = c2.astype(bf16)
    s2 = s2.astype(bf16)

    psw = np.zeros((c.HD, c.HD), np.float32)
    idx = np.arange(c.HD)
    psw[idx, idx ^ 1] = 1.0
    psw = psw.astype(bf16)

    # diagonal-band mask patterns [j][k, q], extracted from the input mask
    qt_last = c.QTN - 1
    q0 = qt_last * c.QT
    maskd = np.empty((c.DIAG, c.KT, c.QT), np.float32)
    for j in range(c.DIAG):
        k0 = (c.DIAG * qt_last + j) * c.KT
        maskd[j] = (mask[0, 0, q0:q0 + c.QT, k0:k0 + c.KT].T == 0.0)
    maskd = maskd.astype(bf16)

    wslices = []
    for hs in range(c.CPG):
        rows = slice(hs * c.DH, (hs + 1) * c.DH)
        wslices.append({
            "wqT": np.ascontiguousarray(wq[rows].T * wsc).astype(pdt),
            "wkT": np.ascontiguousarray(wk[rows].T * wsc).astype(pdt),
            "wvT": np.ascontiguousarray(wv[rows].T * wsc).astype(pdt),
            "wocT": np.ascontiguousarray(wo[:, rows].T).astype(bf16),
        })

    in_maps = []
    for ci in range(c.NCORES):
        b = ci // c.CPG
        hs = ci % c.CPG
        in_maps.append({
            "xT": xTb[b],
            **wslices[hs],
            "waT": waT,
            "c2d": c2,
            "s2d": s2,
            "pswapd": psw,
            "maskdd": maskd,
        })
    return in_maps


def assemble_output(results, cfg: Cfg = FULL) -> np.ndarray:
    c = cfg
    out = np.empty((c.B, c.S, c.DIM), np.float32)
    inv = 1.0 / (WSCALE if c.use_fp8 else 1.0)
    for b in range(c.B):
        total = np.zeros((c.DIM, c.S), np.float32)
        for hs in range(c.CPG):
            total += np.asarray(results[b * c.CPG + hs]["pout"]).astype(np.float32)
        out[b] = total.T * inv
    return out


_NC_CACHE = {}


def run(nc, in_maps, trace=False, cfg: Cfg = FULL, **kw):
    return bass_utils.run_bass_kernel_spmd(
        nc, in_maps, core_ids=list(range(cfg.NCORES)), trace=trace, **kw)


def kernel(**inputs) -> np.ndarray:
    cfg = FULL
    if cfg not in _NC_CACHE:
        _NC_CACHE[cfg] = build_nc(cfg)
    nc = _NC_CACHE[cfg]
    in_maps = make_core_inputs(inputs, cfg)
    res = run(nc, in_maps, cfg=cfg)
    return assemble_output(res.results, cfg)


if __name__ == "__main__":
    nc = build_nc(FULL)
    print("built ok")



# revision 6
# speedup vs baseline: 1.0820x; 1.0820x over previous
"""Trainium2 Bass kernel for nn_Attention_10771777978404 (sparse_attention).

Sharding over 8 NeuronCores: 2 batch-groups x 4 cores (tensor parallel over
heads within each batch group).
  - core ci handles batch ci//4 and heads [4*(ci%4), 4*(ci%4)+4): it computes
    its q/k/v projections (columns of wq/wk/wv), RoPE, causal attention with
    the low-rank sigmoid gate, and a full-width partial of the output
    projection from its 4 heads (rows of wo).
  - the rank-32 adapter weights are replicated inside each batch group; the
    sigmoid gate is computed as 1/(1+exp(-A)) so the scalar engine only ever
    uses the Exp table (no Sigmoid-table reloads, no DRAM staging).
  - host sums the 4 partial output projections per batch (fp16 partials).

Everything on-device is bf16 with fp32 PSUM accumulation.

Schedule notes (v2):
  - diagonal-band tiles only compute the live query columns [128j:512]
    (causal wedge), cutting PE/ACT/DVE work on the band by ~37%.
  - per-head softmax denominators live in ONE PSUM bank at partition
    offsets 32h, removing the head-boundary WAR on the rowsum accumulator.
  - the wo partial-projection PSUM tiles rotate over 4 banks (pp+po pools)
    and the PSUM->SBUF copies alternate ACT/DVE; the per-chunk output DMAs
    are batched into one 2MB DMA per query block (HWDGE descriptor
    generation is a serial ~630ns/dma resource).
  - input DMAs are coarsened and ordered by first-use so the HWDGE queue
    delivers x/wq/wk chunks at PE pace from the start.

self-contained: hardcodes the problem shapes; only needs `concourse` (on
PYTHONPATH in this container) + jax axon devices.
"""

import math
from contextlib import ExitStack
from dataclasses import dataclass

import numpy as np
import ml_dtypes

import concourse.tile as tile
from concourse import bacc
from concourse import mybir
from concourse import bass_utils

BF16 = mybir.dt.bfloat16
F16 = mybir.dt.float16
F32 = mybir.dt.float32
AF = mybir.ActivationFunctionType


@dataclass(frozen=True)
class Cfg:
    B: int = 2
    S: int = 2048
    DIM: int = 2048
    NH: int = 16
    HD: int = 128
    RANK: int = 32
    NCORES: int = 8
    GROUPS: int = 2     # batch groups of 4 cores
    QT: int = 512       # query block (free dim of score tiles)
    KT: int = 128       # key block (partition dim of score tiles)
    PIPE: int = 4       # score tiles in flight ahead of rowsum/AV drains
    repeat: int = 1     # hardware-loop repetitions of the whole body (timing)
    # ablation flags (profiling on hardware; all True for the real kernel)
    use_gate: bool = True
    use_rowsum: bool = True
    use_attn: bool = True
    use_wo: bool = True

    @property
    def CPG(self):
        return self.NCORES // self.GROUPS  # cores per batch group

    @property
    def HLOC(self):
        return self.NH // self.CPG  # heads per core (4)

    @property
    def DH(self):
        return self.HLOC * self.HD  # per-core head-dim span (512)

    @property
    def KTILES(self):
        return self.DIM // 128  # contraction tiles for projections

    @property
    def QTN(self):
        return self.S // self.QT

    @property
    def DIAG(self):
        return self.QT // self.KT  # k-tiles per diagonal band


FULL = Cfg()


def build_nc(cfg: Cfg = FULL):
    c = cfg
    assert c.HD == 128 and c.KT == 128
    nc = bacc.Bacc("TRN2", target_bir_lowering=False, debug=False,
                   num_devices=c.NCORES)

    # ---- kernel I/O (per core: one batch, HLOC heads) ----
    xT = nc.dram_tensor("xT", [c.DIM, c.S], BF16, kind="ExternalInput")
    wqT = nc.dram_tensor("wqT", [c.DIM, c.DH], BF16, kind="ExternalInput")
    wkT = nc.dram_tensor("wkT", [c.DIM, c.DH], BF16, kind="ExternalInput")
    wvT = nc.dram_tensor("wvT", [c.DIM, c.DH], BF16, kind="ExternalInput")
    wocT = nc.dram_tensor("wocT", [c.DH, c.DIM], BF16, kind="ExternalInput")
    waT = nc.dram_tensor("waT", [c.DIM, 2 * c.RANK], BF16, kind="ExternalInput")
    c2d = nc.dram_tensor("c2d", [c.HD, c.S], BF16, kind="ExternalInput")
    s2d = nc.dram_tensor("s2d", [c.HD, c.S], BF16, kind="ExternalInput")
    pswapd = nc.dram_tensor("pswapd", [c.HD, c.HD], BF16, kind="ExternalInput")
    maskdd = nc.dram_tensor("maskdd", [c.DIAG, c.KT, c.QT], BF16, kind="ExternalInput")

    # partial output projection, transposed: pout[j, t] (fp16; host sums the
    # 4 partials of each batch group in fp32)
    pout = nc.dram_tensor("pout", [c.DIM, c.S], F16, kind="ExternalOutput")

    isqrt = 1.0 / math.sqrt(c.HD)
    sc_score = isqrt
    sc_gate = -1.0
    NKT = c.KTILES

    with ExitStack() as _ctx:
        tc = _ctx.enter_context(tile.TileContext(nc))
        # persistent pools (whole-iteration lifetime)
        cst = _ctx.enter_context(tc.tile_pool(name="const", bufs=1))
        adp = _ctx.enter_context(tc.tile_pool(name="ap", bufs=1))
        qkp = _ctx.enter_context(tc.tile_pool(name="qk", bufs=1))
        vp = _ctx.enter_context(tc.tile_pool(name="vp", bufs=1))
        rtp = _ctx.enter_context(tc.tile_pool(name="rope_t", bufs=1))
        pge = _ctx.enter_context(tc.tile_pool(name="pge", bufs=5))
        gwk = _ctx.enter_context(tc.tile_pool(name="gwk", bufs=2))
        ogp = _ctx.enter_context(tc.tile_pool(name="og", bufs=1))
        # PSUM pools (8 banks total): pp 2 + ps 4 + po 2; the per-head
        # softmax-denominator tiles and the norm-broadcast tiles share the
        # pp rotation so rowsum accumulation never WARs the previous head
        pp = _ctx.enter_context(tc.tile_pool(name="pp", bufs=2, space="PSUM"))
        psp = _ctx.enter_context(tc.tile_pool(name="ps", bufs=4, space="PSUM"))
        pop = _ctx.enter_context(tc.tile_pool(name="po", bufs=2, space="PSUM"))

        def body():
            # ---- constants ----
            c2_sb = cst.tile([128, c.S], BF16, name="c2_sb", tag="c2")
            s2_sb = cst.tile([128, c.S], BF16, name="s2_sb", tag="s2")
            psw_sb = cst.tile([128, 128], BF16, name="psw_sb", tag="psw")
            mask_sb = cst.tile([128, c.DIAG, c.QT], BF16, name="mask_sb", tag="mask")
            ones_sb = cst.tile([128, 1], BF16, name="ones_sb", tag="ones")
            oner_sb = cst.tile([1, 128], F16, name="oner_sb", tag="oner")

            # packed adapter projections: one [64,512] matmul computes both
            # aq and ak rows (halves the adapter matmul count); ak is then
            # moved to partition base 0 with one SBUF->SBUF DMA (engines
            # cannot shift partitions, DMA can)
            aqk_sb = adp.tile([2 * c.RANK, c.S], BF16, name="aqk_sb", tag="aqk")
            ak_sb = adp.tile([c.RANK, c.S], BF16, name="ak_sb", tag="ak")
            q_sb = [qkp.tile([128, c.S], BF16, name=f"q{h}_sb", tag=f"q{h}")
                    for h in range(c.HLOC)]
            k_sb = [qkp.tile([128, c.S], BF16, name=f"k{h}_sb", tag=f"k{h}")
                    for h in range(c.HLOC)]
            v_sb = vp.tile([128, c.S // 128, c.DH], BF16, name="v_sb", tag="v")

            with tc.tile_pool(name="xtp", bufs=1) as xtp:
                wa_sb = xtp.tile([128, c.KTILES, 2 * c.RANK], BF16,
                                 name="wa_sb", tag="wa")
                nc.sync.dma_start(out=wa_sb,
                                  in_=waT.ap().rearrange("(t p) m -> p t m", p=128))
                xt_sb = xtp.tile([128, c.KTILES, c.S], BF16, name="xt_sb", tag="xt")
                xr = xT.ap().rearrange("(t p) n -> p t n", p=128)

                def xsl(j, sl):
                    return xt_sb[:, j, sl]

                nc.vector.memset(ones_sb, 1.0)
                nc.vector.memset(oner_sb, 1.0)

                # ---- adapter + q/k projections, kt-outer, 8 live psum accs ----
                apools = [pp, psp, pop, psp]
                acc = [apools[i].tile([2 * c.RANK, c.QT], F32, name=f"acc_a{i}",
                                      tag=("pp", "ps", "po", "ps")[i])
                       for i in range(c.QTN)]

                with tc.tile_pool(name="wqk", bufs=1) as wp:
                    wq_sb = wp.tile([128, c.KTILES, c.DH], BF16, name="wq_sb", tag="wq")
                    wk_sb = wp.tile([128, c.KTILES, c.DH], BF16, name="wk_sb", tag="wk")
                    # input DMAs ordered by first use; chunk sizes grow so the
                    # HWDGE queue stays ahead of the PE's kt-outer consumption
                    nc.sync.dma_start(out=xt_sb[:, 0, :], in_=xr[:, 0, :])
                    wqr = wqT.ap().rearrange("(t p) m -> p t m", p=128)
                    wkr = wkT.ap().rearrange("(t p) m -> p t m", p=128)
                    nc.scalar.dma_start(out=wq_sb[:, 0:4, :], in_=wqr[:, 0:4, :])
                    nc.scalar.dma_start(out=wk_sb[:, 0:4, :], in_=wkr[:, 0:4, :])
                    nc.sync.dma_start(out=xt_sb[:, 1, :], in_=xr[:, 1, :])
                    nc.sync.dma_start(out=xt_sb[:, 2:4, :], in_=xr[:, 2:4, :])
                    nc.scalar.dma_start(out=wq_sb[:, 4:, :], in_=wqr[:, 4:, :])
                    nc.scalar.dma_start(out=wk_sb[:, 4:, :], in_=wkr[:, 4:, :])
                    nc.sync.dma_start(out=xt_sb[:, 4:8, :], in_=xr[:, 4:8, :])
                    nc.sync.dma_start(out=xt_sb[:, 8:12, :], in_=xr[:, 8:12, :])
                    nc.sync.dma_start(out=xt_sb[:, 12:16, :], in_=xr[:, 12:16, :])

                    allqk = [(dst, w, h, qt)
                             for dst, w in ((q_sb, wq_sb), (k_sb, wk_sb))
                             for h in range(c.HLOC)
                             for qt in range(c.QTN)]
                    # first four groups run kt-outer interleaved with the
                    # adapter accumulation so the PE issues 8 matmuls per
                    # arriving x chunk instead of 4
                    head_pools = [(pp, "pp"), (psp, "ps"), (psp, "ps"),
                                  (pop, "po")]
                    headacc = [head_pools[i][0].tile([128, c.QT], F32,
                                                     name=f"acc_qk{i}",
                                                     tag=head_pools[i][1])
                               for i in range(4)]
                    for kt in range(NKT):
                        for qt in range(c.QTN):
                            nc.tensor.matmul(
                                acc[qt][:, :],
                                wa_sb[:, kt, :],
                                xsl(kt, slice(qt * c.QT, (qt + 1) * c.QT)),
                                start=(kt == 0), stop=(kt == NKT - 1))
                        for i in range(4):
                            dst, w, h, qt = allqk[i]
                            nc.tensor.matmul(
                                headacc[i][:, :],
                                w[:, kt, h * 128:(h + 1) * 128],
                                xsl(kt, slice(qt * c.QT, (qt + 1) * c.QT)),
                                start=(kt == 0), stop=(kt == NKT - 1))
                    for qt in range(c.QTN):
                        nc.vector.tensor_copy(
                            aqk_sb[:, qt * c.QT:(qt + 1) * c.QT], acc[qt][:, :])
                    nc.sync.dma_start(out=ak_sb[:, :],
                                      in_=aqk_sb[c.RANK:2 * c.RANK, :])
                    for i in range(4):
                        dst, w, h, qt = allqk[i]
                        nc.scalar.copy(dst[h][:, qt * c.QT:(qt + 1) * c.QT],
                                       headacc[i][:, :])
                    for dst, w, h, qt in allqk[4:]:
                        psum = pp.tile([128, c.QT], F32, name="psum_qk", tag="pp")
                        for kt in range(NKT):
                            nc.tensor.matmul(
                                psum[:, :],
                                w[:, kt, h * 128:(h + 1) * 128],
                                xsl(kt, slice(qt * c.QT, (qt + 1) * c.QT)),
                                start=(kt == 0), stop=(kt == NKT - 1))
                        nc.scalar.copy(dst[h][:, qt * c.QT:(qt + 1) * c.QT],
                                       psum[:, :])

                # ---- v projection: [tok, d] natural, moving 512 wide ----
                # RoPE tiles are interleaved into the v loop: the rope chain
                # is DVE/GPSIMD-paced, the v matmuls keep the PE busy
                rope_tiles = [(tiles, h, qt)
                              for tiles in (q_sb, k_sb)
                              for h in range(c.HLOC)
                              for qt in range(c.QTN)]

                def rope_step(i):
                    # m1 on GPSIMD (SBUF-only engine), m2 on DVE (PSUM read),
                    # final add alternates so neither engine is the pacer
                    tiles, h, qt = rope_tiles[i]
                    eng = nc.vector if i % 2 == 0 else nc.gpsimd
                    sl = slice(qt * c.QT, (qt + 1) * c.QT)
                    pswp = psp.tile([128, c.QT], F32, name="pswp", tag="ps")
                    nc.tensor.matmul(pswp[:, :], psw_sb[:, :],
                                     tiles[h][:, sl], start=True, stop=True)
                    m1 = rtp.tile([128, c.QT], BF16, name="rope_m1",
                                  tag=f"m1{i % 2}")
                    m2 = rtp.tile([128, c.QT], BF16, name="rope_m2",
                                  tag=f"m2{i % 2}")
                    nc.gpsimd.tensor_mul(m1[:, :], tiles[h][:, sl], c2_sb[:, sl])
                    nc.vector.tensor_mul(m2[:, :], pswp[:, :], s2_sb[:, sl])
                    eng.tensor_add(tiles[h][:, sl], m1[:, :], m2[:, :])

                with tc.tile_pool(name="wvp", bufs=1) as wvpool:
                    wv_sb = wvpool.tile([128, c.KTILES, c.DH], BF16,
                                        name="wv_sb", tag="wv")
                    wr = wvT.ap().rearrange("(t p) m -> p t m", p=128)
                    nc.sync.dma_start(out=wv_sb[:, 0:8, :], in_=wr[:, 0:8, :])
                    nc.sync.dma_start(out=wv_sb[:, 8:16, :], in_=wr[:, 8:16, :])
                    # rope tables arrive while the first v token-blocks run
                    nc.sync.dma_start(out=c2_sb, in_=c2d.ap())
                    nc.sync.dma_start(out=s2_sb, in_=s2d.ap())
                    nc.sync.dma_start(out=psw_sb, in_=pswapd.ap())
                    for tt in range(c.S // 128):
                        psum = pp.tile([128, c.DH], F32, name="psum_v", tag="pp")
                        for kt in range(NKT):
                            nc.tensor.matmul(
                                psum[:, :],
                                xsl(kt, slice(tt * 128, (tt + 1) * 128)),
                                wv_sb[:, kt, :],
                                start=(kt == 0), stop=(kt == NKT - 1))
                        nc.scalar.copy(v_sb[:, tt, :], psum[:, :])
                        rope_step(2 * tt)
                        rope_step(2 * tt + 1)

            # ---- attention + output projection, per 512-query block ----
            with tc.tile_pool(name="wog", bufs=1) as wog, \
                    tc.tile_pool(name="wo_out", bufs=2) as wop:
                nc.sync.dma_start(out=mask_sb,
                                  in_=maskdd.ap().rearrange("j p q -> p j q"))
                woc_sb = wog.tile([128, c.HLOC, c.DIM], BF16, name="woc_sb", tag="woc")
                wcr = wocT.ap().rearrange("(h p) j -> p h j", p=128)
                nc.sync.dma_start(out=woc_sb[:, 0:2, :], in_=wcr[:, 0:2, :])
                nc.sync.dma_start(out=woc_sb[:, 2:4, :], in_=wcr[:, 2:4, :])
                # double-buffered per-qt gate tiles r = sigmoid(A)[k, q]
                rg_sb = [wog.tile([128, c.DIAG * c.QTN, c.QT], BF16,
                                  name=f"rg{i}_sb", tag=f"rg{i}") for i in range(2)]

                def gate_step(qt, kt):
                    # r = 1/(1+exp(-A)) via the (shared) Exp table
                    qsl = slice(qt * c.QT, (qt + 1) * c.QT)
                    ksl = slice(kt * c.KT, (kt + 1) * c.KT)
                    pga = psp.tile([128, c.QT], F32, name="pga", tag="ps")
                    nc.tensor.matmul(pga[:, :], ak_sb[:, ksl],
                                     aqk_sb[0:c.RANK, qsl],
                                     start=True, stop=True)
                    ge = pge.tile([128, c.QT], BF16, name="ge", tag="p")
                    nc.scalar.activation(ge[:, :], pga[:, :], AF.Exp, scale=sc_gate)
                    gt = gwk.tile([128, c.QT], F32, name="gt", tag="gt")
                    nc.vector.tensor_scalar_add(gt[:, :], ge[:, :], 1.0)
                    gr = gwk.tile([128, c.QT], F32, name="gr", tag="gr")
                    nc.vector.reciprocal_approx_fast(out=gr[:, :], in_=gt[:, :])
                    nc.gpsimd.tensor_copy(rg_sb[qt % 2][:, kt, :], gr[:, :])

                if c.use_gate and c.use_attn:
                    for kt in range(c.DIAG):
                        gate_step(0, kt)

                for qt in range(c.QTN):
                    qsl = slice(qt * c.QT, (qt + 1) * c.QT)
                    nkt = c.DIAG * (qt + 1)  # causal k tiles
                    rg = rg_sb[qt % 2]

                    og_sb = ogp.tile([128, c.HLOC, c.QT], BF16, name="og_sb", tag="og")
                    if not c.use_attn:
                        nc.vector.memset(og_sb, 0.0)
                    # normalize chain of head h is emitted early in head h+1's
                    # score phase (at kt==2, before h+1's first drain at
                    # kt==PIPE+1) so prs/rbc pp-slot rotation never stalls
                    pending_norm = [None]

                    def make_normalize(h, po, prs):
                        def norm():
                            if not c.use_rowsum:
                                nc.vector.tensor_copy(og_sb[:, h, :], po[:, :])
                                return
                            rr = gwk.tile([1, c.QT], F32, name="rr", tag="rr")
                            nc.vector.reciprocal_approx_fast(
                                out=rr[:, :], in_=prs[0:1, :])
                            rrh = gwk.tile([1, c.QT], F16, name="rrh", tag="rrh")
                            nc.vector.tensor_copy(rrh[:, :], rr[:, :])
                            rbc = pp.tile([128, c.QT], F32, name="rbc", tag="pp")
                            nc.tensor.matmul(rbc[:, :], oner_sb[:, :], rrh[:, :],
                                             start=True, stop=True)
                            # DVE can't take two PSUM operands; stage the
                            # broadcast in SBUF first
                            rbs = gwk.tile([128, c.QT], F16, name="rbs", tag="rbs")
                            nc.vector.tensor_copy(rbs[:, :], rbc[:, :])
                            nc.vector.tensor_mul(og_sb[:, h, :], po[:, :],
                                                 rbs[:, :])
                        return norm

                    for h in range(c.HLOC if c.use_attn else 0):
                        po = pop.tile([128, c.QT], F32, name="po", tag="po")
                        prs = pp.tile([1, c.QT], F32, name="prs", tag="pp")

                        # score matmuls run PIPE steps ahead of the dependent
                        # rowsum/AV matmuls so the exp/mask/gate chain latency
                        # stays off the in-order PE queue
                        stage = []  # (kt, col-slice, p_or_pm_tile, pgm_tile)

                        def score_step(kt):
                            ksl = slice(kt * c.KT, (kt + 1) * c.KT)
                            j = kt - c.DIAG * qt
                            qoff = 128 * j if j > 0 else 0
                            s = slice(qoff, c.QT)
                            qs = slice(qt * c.QT + qoff, (qt + 1) * c.QT)
                            ps = psp.tile([128, c.QT], F32, name="ps", tag="ps")
                            nc.tensor.matmul(ps[:, s], k_sb[h][:, ksl],
                                             q_sb[h][:, qs], start=True, stop=True)
                            p_sb = pge.tile([128, c.QT], BF16, name="p_sb", tag="p")
                            nc.scalar.activation(p_sb[:, s], ps[:, s], AF.Exp,
                                                 scale=sc_score)
                            if j >= 0:
                                # diagonal band: 0/1 causal mask after exp
                                pm = pge.tile([128, c.QT], BF16, name="pm", tag="pm")
                                nc.vector.tensor_mul(pm[:, s], p_sb[:, s],
                                                     mask_sb[:, j, s])
                                p_sb = pm
                            if c.use_gate:
                                pgm = pge.tile([128, c.QT], BF16, name="pgm",
                                               tag="pgm")
                                nc.vector.tensor_mul(pgm[:, s], p_sb[:, s],
                                                     rg[:, kt, s])
                            else:
                                pgm = p_sb
                            stage.append((kt, s, p_sb, pgm))

                        def drain_step():
                            kt, s, p_sb, pgm = stage.pop(0)
                            # pre-gate rowsum (softmax denominator)
                            if c.use_rowsum:
                                nc.tensor.matmul(prs[0:1, s],
                                                 ones_sb[:, :], p_sb[:, s],
                                                 start=(kt == 0),
                                                 stop=(kt == nkt - 1),
                                                 skip_group_check=True)
                            # out_h^T[d, q] += v[k, d].T @ p_gated[k, q]
                            nc.tensor.matmul(po[:, s],
                                             v_sb[:, kt, h * 128:(h + 1) * 128],
                                             pgm[:, s],
                                             start=(kt == 0), stop=(kt == nkt - 1),
                                             skip_group_check=True)

                        for kt in range(nkt):
                            score_step(kt)
                            if kt == min(nkt - 1, 2) and pending_norm[0] is not None:
                                pending_norm[0]()
                                pending_norm[0] = None
                            if len(stage) > c.PIPE:
                                drain_step()
                        while stage:
                            drain_step()
                        assert pending_norm[0] is None, "normalize dropped"
                        pending_norm[0] = make_normalize(h, po, prs)
                    # last head's normalize must land before the wo matmuls
                    if pending_norm[0] is not None:
                        pending_norm[0]()

                    # output-projection partial for this query block,
                    # interleaved with the NEXT block's gate generation so the
                    # PE fills the gate chain's latency with wo matmuls
                    nkt2 = (c.DIAG * (qt + 2)
                            if (qt + 1 < c.QTN and c.use_gate and c.use_attn)
                            else 0)
                    ncha = c.DIM // 128
                    f_sb = wop.tile([128, ncha, c.QT], F16, name="f_sb", tag="f")
                    pfpools = [(pp, "pp"), (pop, "po")]
                    for ch in range(ncha if c.use_wo else 0):
                        pfp, pft = pfpools[ch % 2]
                        pf = pfp.tile([128, c.QT], F32, name="pf", tag=pft)
                        for h in range(c.HLOC):
                            nc.tensor.matmul(
                                pf[:, :],
                                woc_sb[:, h, ch * 128:(ch + 1) * 128],
                                og_sb[:, h, :],
                                start=(h == 0), stop=(h == c.HLOC - 1))
                        if ch % 2 == 0:
                            nc.scalar.copy(f_sb[:, ch, :], pf[:, :])
                        else:
                            nc.vector.tensor_copy(f_sb[:, ch, :], pf[:, :])
                        if ch < nkt2:
                            gate_step(qt + 1, ch)
                        if ch % 4 == 3:
                            # batched output DMA per 4 chunks (0.5MB each):
                            # early chunks fly while later ones compute
                            nc.sync.dma_start(
                                out=pout.ap().rearrange(
                                    "(ch p) q -> p ch q",
                                    p=128)[:, ch - 3:ch + 1, qsl],
                                in_=f_sb[:, ch - 3:ch + 1, :])
                    if not c.use_wo:
                        for kt in range(nkt2):
                            gate_step(qt + 1, kt)

        if c.repeat > 1:
            with tc.For_i(0, c.repeat, 1,
                          hint_engines=(mybir.EngineType.PE,
                                        mybir.EngineType.DVE,
                                        mybir.EngineType.Activation,
                                        mybir.EngineType.Pool,
                                        mybir.EngineType.SP)):
                body()
        else:
            body()

    nc.compile()
    return nc


def make_core_inputs(inputs: dict, cfg: Cfg = FULL):
    """Host-side sharding: returns in_maps (one dict per core)."""
    c = cfg
    bf16 = ml_dtypes.bfloat16
    x = np.asarray(inputs["x"])
    mask = np.asarray(inputs["mask"])
    fc = np.asarray(inputs["freqs_cos"])
    fs = np.asarray(inputs["freqs_sin"])
    wq, wk, wv, wo = (np.asarray(inputs[k]) for k in ("wq", "wk", "wv", "wo"))
    wa_q, wa_k = np.asarray(inputs["wa_q"]), np.asarray(inputs["wa_k"])

    xTb = [np.ascontiguousarray(x[b].T).astype(bf16) for b in range(c.B)]
    waT = np.ascontiguousarray(
        np.concatenate([wa_q, wa_k], axis=0).T).astype(bf16)

    # rope tables in [d, tok] layout
    c2 = np.empty((c.HD, c.S), np.float32)
    s2 = np.empty((c.HD, c.S), np.float32)
    c2[0::2] = fc.T
    c2[1::2] = fc.T
    s2[0::2] = -fs.T
    s2[1::2] = fs.T
    c2 = c2.astype(bf16)
    s2 = s2.astype(bf16)

    psw = np.zeros((c.HD, c.HD), np.float32)
    idx = np.arange(c.HD)
    psw[idx, idx ^ 1] = 1.0
    psw = psw.astype(bf16)

    # diagonal-band mask patterns [j][k, q], extracted from the input mask
    qt_last = c.QTN - 1
    q0 = qt_last * c.QT
    maskd = np.empty((c.DIAG, c.KT, c.QT), np.float32)
    for j in range(c.DIAG):
        k0 = (c.DIAG * qt_last + j) * c.KT
        maskd[j] = (mask[0, 0, q0:q0 + c.QT, k0:k0 + c.KT].T == 0.0)
    maskd = maskd.astype(bf16)

    wslices = []
    for hs in range(c.CPG):
        rows = slice(hs * c.DH, (hs + 1) * c.DH)
        wslices.append({
            "wqT": np.ascontiguousarray(wq[rows].T).astype(bf16),
            "wkT": np.ascontiguousarray(wk[rows].T).astype(bf16),
            "wvT": np.ascontiguousarray(wv[rows].T).astype(bf16),
            "wocT": np.ascontiguousarray(wo[:, rows].T).astype(bf16),
        })

    in_maps = []
    for ci in range(c.NCORES):
        b = ci // c.CPG
        hs = ci % c.CPG
        in_maps.append({
            "xT": xTb[b],
            **wslices[hs],
            "waT": waT,
            "c2d": c2,
            "s2d": s2,
            "pswapd": psw,
            "maskdd": maskd,
        })
    return in_maps


def assemble_output(results, cfg: Cfg = FULL) -> np.ndarray:
    c = cfg
    out = np.empty((c.B, c.S, c.DIM), np.float32)
    for b in range(c.B):
        total = np.zeros((c.DIM, c.S), np.float32)
        for hs in range(c.CPG):
            total += np.asarray(results[b * c.CPG + hs]["pout"]).astype(np.float32)
        out[b] = total.T
    return out


_NC_CACHE = {}


def run(nc, in_maps, trace=False, cfg: Cfg = FULL, **kw):
    return bass_utils.run_bass_kernel_spmd(
        nc, in_maps, core_ids=list(range(cfg.NCORES)), trace=trace, **kw)


def kernel(**inputs) -> np.ndarray:
    cfg = FULL
    if cfg not in _NC_CACHE:
        _NC_CACHE[cfg] = build_nc(cfg)
    nc = _NC_CACHE[cfg]
    in_maps = make_core_inputs(inputs, cfg)
    res = run(nc, in_maps, cfg=cfg)
    return assemble_output(res.results, cfg)


if __name__ == "__main__":
    nc = build_nc(FULL)
    print("built ok")
